# revision 1
# baseline (speedup 1.0000x reference)
"""Trainium2 Bass kernel for MultiHeadAttention with relative position bias.

Reference computation (B=2, S=2048, D=1024, H=16, Dk=64, MAX_REL=128):
    Q,K,V = x@W{q,k,v}.T + b      (per-head reshape)
    scores = QK^T/sqrt(Dk) + rel_bias_matrix
    out = softmax(scores) @ V, heads merged, @ Wo.T + bo

Sharding (8 cores): core c handles batch b=c//4 and 4 heads hg=4*(c%4)..+4
(data + head parallel). Q/K/V projections column-split per head group,
Wo row-split; the partial outputs are summed on the host (the "all-reduce").

Per-core device algorithm (channels-on-partitions transposed layouts):
  xT (1024,2048) -> Q^T,K^T (c_local=256, S) on PE;  V as (S, dv=256).
  Per head pair (row-tiled 64x128 PE, two heads concurrent):
    S^T[k,q] = K^T.T @ Q^T, then P^T = exp(S^T/8 + bias) via one ACT pass
    with bias=rel_bias[h,256] ("past" constant) folded in; the "future"
    region (q-k <= -128) is fixed up with a constant multiply and the
    384-wide Toeplitz band around the diagonal with a host-precomputed
    exp(bias - c_past) tile (both on DVE).  P^T stored in bf16.
  PV: acc[dv+1, q] += V_aug.T@P^T with a ones column on V giving the
  softmax denominator for free; normalize via reciprocal + gpsimd
  partition-broadcast; Wo partial = C^T.T @ (Wo^T rows).
"""

import math
import os
import sys

for _p in ("/opt/trn_rl_repo", "/root/.axon_site", "/root/.axon_site/_ro/trn_rl_repo",
           "/root/.axon_site/_ro/pypackages"):
    if os.path.isdir(_p) and _p not in sys.path:
        sys.path.append(_p)

import numpy as np
import ml_dtypes

import concourse.bass as bass
import concourse.mybir as mybir
import concourse.tile as tile
from concourse import bacc, library_config
from contextlib import ExitStack

# Problem constants (hardcoded per the contract).
B, S, D = 2, 2048, 1024
H, DK = 16, 64
MAX_REL = 128
N_CORES = 8
CORES_PER_BATCH = 4
HEADS_PER_CORE = H // CORES_PER_BATCH  # 4
CL = HEADS_PER_CORE * DK               # 256 local channels
N_PAIRS = HEADS_PER_CORE // 2          # 2 head pairs
QH = 1024                              # q processed in halves
N_QH = S // QH                         # 2
N_KC = S // 128                        # 16 k chunks
BAND = 3 * 128                         # band width in q for one k chunk

F32 = mybir.dt.float32
F32R = mybir.dt.float32r
BF16 = mybir.dt.bfloat16

SCALE = 1.0 / math.sqrt(DK)

EXP = mybir.ActivationFunctionType.Exp


def r32(ap):
    return ap  # tiles are declared float32r natively


def build_program(matmul_f32r=True, p_bf16=True, reps=1, interleave_pv=True,
                  skip_norm=False):
    nc = bacc.Bacc("TRN2", target_bir_lowering=False, debug=False)

    mm = r32 if matmul_f32r else (lambda ap: ap)
    PD = BF16 if p_bf16 else F32

    xt_d = nc.declare_dram_parameter("xt", [D, S], F32R, isOutput=False)
    wqt_d = nc.declare_dram_parameter("wqt", [D, CL], F32R, isOutput=False)
    wkt_d = nc.declare_dram_parameter("wkt", [D, CL], F32R, isOutput=False)
    wvt_d = nc.declare_dram_parameter("wvt", [D, CL], F32R, isOutput=False)
    wot_d = nc.declare_dram_parameter("wot", [CL, D], F32R, isOutput=False)
    bqk_d = nc.declare_dram_parameter("bqk", [128, 4], F32, isOutput=False)
    band_d = nc.declare_dram_parameter("band", [128, HEADS_PER_CORE, BAND], F32,
                                       isOutput=False)
    # per-head activation constants, replicated over partitions:
    # [:, 2h] = exp(c_fut - c_past) multiplier, [:, 2h+1] = c_past bias
    abias_d = nc.declare_dram_parameter("abias", [128, 2 * HEADS_PER_CORE], F32,
                                        isOutput=False)
    out_d = nc.declare_dram_parameter("out_p", [S, D], F32, isOutput=True)
    # scratch for the denominator broadcast: [hh, pair*qh, q]
    den_d = nc.dram_tensor("den_scratch", [1, 2 * N_PAIRS * N_QH, QH], F32)

    with tile.TileContext(nc) as tc, ExitStack() as ctx:
        # ---------- long-lived SBUF ----------
        persist = ctx.enter_context(tc.tile_pool(name="persist", bufs=1))
        q_sb = persist.tile([128, 2, S], F32R, tag="q_sb")
        k_sb = persist.tile([128, 2, S], F32R, tag="k_sb")
        v_sb = persist.tile([128, N_KC, HEADS_PER_CORE, DK + 1], PD, tag="v_sb")
        ct_sb = persist.tile([128, 2, S], F32R, tag="ct_sb")
        wo_sb = persist.tile([128, 2, D], F32R, tag="wo_sb")
        band_sb = persist.tile([128, HEADS_PER_CORE, BAND], PD, tag="band_sb")
        bqk_sb = persist.tile([128, 4], F32, tag="bqk_sb")
        abias_sb = persist.tile([128, 2 * HEADS_PER_CORE], F32, tag="abias_sb")

        nc.gpsimd.load_library(library_config.attn)
        nc.sync.dma_start(out=wo_sb, in_=wot_d.ap().rearrange("(c p) m -> p c m", p=128))
        nc.sync.dma_start(out=bqk_sb, in_=bqk_d.ap())
        nc.sync.dma_start(out=abias_sb, in_=abias_d.ap())
        if p_bf16:
            band_f32 = persist.tile([128, HEADS_PER_CORE, BAND], F32, tag="band_f32")
            nc.sync.dma_start(out=band_f32, in_=band_d.ap())
            nc.vector.tensor_copy(out=band_sb, in_=band_f32)
        else:
            nc.sync.dma_start(out=band_sb, in_=band_d.ap())

        # ---------- PSUM pools ----------
        stp = ctx.enter_context(tc.tile_pool(name="stp", bufs=2, space="PSUM"))
        accp = ctx.enter_context(tc.tile_pool(name="accp", bufs=2, space="PSUM"))

        sb = dict(q=q_sb, k=k_sb, v=v_sb, ct=ct_sb, wo=wo_sb, band=band_sb,
                  bqk=bqk_sb, abias=abias_sb)
        dram = dict(xt=xt_d, wqt=wqt_d, wkt=wkt_d, wvt=wvt_d, out=out_d, den=den_d)

        outp = ctx.enter_context(tc.tile_pool(name="outp", bufs=4))
        pools = dict(stp=stp, accp=accp, outp=outp)

        for rep in range(reps):
            _phases(nc, tc, mm, PD, sb, dram, pools, rep,
                    interleave_pv=interleave_pv, skip_norm=skip_norm)

    nc.compile()
    return nc


def _phases(nc, tc, mm, PD, sb, dram, pools, rep, interleave_pv=True,
            skip_norm=False):
    q_sb, k_sb, v_sb, ct_sb, wo_sb = sb["q"], sb["k"], sb["v"], sb["ct"], sb["wo"]
    band_sb, bqk_sb, abias_sb = sb["band"], sb["bqk"], sb["abias"]
    stp, accp, outp = (pools[n] for n in ("stp", "accp", "outp"))

    # ---------- phase 1: projections ----------
    with ExitStack() as proj_ctx:
        xw = proj_ctx.enter_context(tc.tile_pool(name=f"xw{rep}", bufs=1))
        xt_sb = xw.tile([128, D // 128, S], F32R, tag="xt_sb")
        wq_sb = xw.tile([128, D // 128, CL], F32R, tag="wq_sb")
        wk_sb = xw.tile([128, D // 128, CL], F32R, tag="wk_sb")
        wv_sb = xw.tile([128, D // 128, CL], F32R, tag="wv_sb")

        xt_v = dram["xt"].ap().rearrange("(c p) s -> p c s", p=128)
        for w_sb, d_t in ((wq_sb, dram["wqt"]), (wk_sb, dram["wkt"]), (wv_sb, dram["wvt"])):
            nc.sync.dma_start(out=w_sb, in_=d_t.ap().rearrange("(c p) m -> p c m", p=128))
        for dc in range(D // 128):
            nc.sync.dma_start(out=xt_sb[:, dc, :], in_=xt_v[:, dc, :])

        # Projections in flights of 4 concurrent PSUM accumulations, with
        # the D-contraction split in 2 chunk groups for DMA/compute overlap.
        NDC = D // 128
        GROUPS = (range(0, NDC // 2), range(NDC // 2, NDC))

        def q_or_k_flight(w_sb, o_sb, boff):
            slot0 = stp.tile([128, 1024], F32, tag="st")
            slot1 = stp.tile([128, 1024], F32, tag="st")
            slot2 = accp.tile([128, 1024], F32, tag="acc")
            slot3 = accp.tile([128, 1024], F32, tag="acc")
            slots = [slot0, slot1, slot2, slot3]
            for g in GROUPS:
                for j in range(2):
                    for t in range(2):
                        ps = slots[j * 2 + t]
                        for half in range(2):
                            for dc in g:
                                nc.tensor.matmul(
                                    ps[:, half * 512:(half + 1) * 512],
                                    lhsT=mm(w_sb[:, dc, j * 128:(j + 1) * 128]),
                                    rhs=mm(xt_sb[:, dc, t * 1024 + half * 512:
                                                 t * 1024 + (half + 1) * 512]),
                                    start=(dc == 0), stop=(dc == NDC - 1),
                                )
            for j in range(2):
                for t in range(2):
                    nc.scalar.add(
                        out=o_sb[:, j, t * 1024:(t + 1) * 1024],
                        in_=slots[j * 2 + t],
                        add=bqk_sb[:, boff + j:boff + j + 1],
                    )

        q_or_k_flight(wq_sb, q_sb, 0)
        q_or_k_flight(wk_sb, k_sb, 2)

        # V : [s_chunk, dv], 4 s-chunks packed per 2 PSUM slots
        for scg in range(N_KC // 4):
            ps = stp.tile([128, 1024], F32, tag="st")
            psb = accp.tile([128, 1024], F32, tag="acc")
            both = (ps, psb)
            for g in GROUPS:
                for i in range(4):
                    sc = scg * 4 + i
                    tgt = both[i // 2]
                    col = (i % 2) * 512
                    for dc in g:
                        nc.tensor.matmul(
                            tgt[:, col:col + CL],
                            lhsT=mm(xt_sb[:, dc, sc * 128:(sc + 1) * 128]),
                            rhs=mm(wv_sb[:, dc, :]),
                            start=(dc == 0), stop=(dc == NDC - 1),
                        )
            for i in range(4):
                sc = scg * 4 + i
                tgt = both[i // 2]
                col = (i % 2) * 512
                nc.scalar.copy(
                    out=v_sb[:, sc, :, 0:DK],
                    in_=tgt[:, col:col + CL].rearrange("p (h d) -> p h d",
                                                       h=HEADS_PER_CORE),
                )
        nc.vector.memset(v_sb[:, :, :, DK:DK + 1], 1.0)

    # ---------- phase 2: attention ----------
    # Note: P~ = exp(s/8) * band/future multipliers is the true softmax
    # numerator up to a constant per-head factor e^{-c_past}, which cancels
    # exactly in the normalization - no bias needed in the exp.
    attn_ctx = ExitStack()
    nrm = attn_ctx.enter_context(tc.tile_pool(name=f"nrm{rep}", bufs=1))
    if interleave_pv:
        ptp = attn_ctx.enter_context(tc.tile_pool(name=f"ptp{rep}", bufs=6))
    else:
        ptp = attn_ctx.enter_context(tc.tile_pool(name=f"ptp{rep}", bufs=1))
        pt_all = ptp.tile([128, 2, N_KC, QH], PD, tag="pt_all")

    def qk_exp_fix(pair, hh, kc, w0, st, pt_dst):
        """QK matmuls + exp + band/future fixups for one (head, chunk)."""
        k0 = kc * 128
        h = 2 * pair + hh
        p0 = hh * 64
        for half in range(QH // 512):
            nc.tensor.matmul(
                st[:, half * 512:(half + 1) * 512],
                lhsT=mm(k_sb[p0:p0 + 64, pair, k0:k0 + 128]),
                rhs=mm(q_sb[p0:p0 + 64, pair,
                            w0 + half * 512:w0 + (half + 1) * 512]),
                start=True, stop=True,
                tile_position=(p0, 0),
            )
        nc.scalar.activation(out=pt_dst, in_=st, func=EXP, scale=SCALE)
        # future region (q <= k0-129): multiply by exp(c_fut - c_past)
        fut_end = min(max(k0 - 128, w0), w0 + QH)
        n_fut = fut_end - w0
        if n_fut > 0:
            nc.vector.tensor_scalar_mul(
                out=pt_dst[:, 0:n_fut], in0=pt_dst[:, 0:n_fut],
                scalar1=abias_sb[:, 2 * h:2 * h + 1],
            )
        # band: q in [k0-128, k0+256) -> multiply exp(bias - c_past)
        b_lo = max(k0 - 128, w0)
        b_hi = min(k0 + 2 * 128, w0 + QH)
        if b_hi > b_lo:
            m0 = b_lo - (k0 - 128)
            nc.vector.tensor_mul(
                out=pt_dst[:, b_lo - w0:b_hi - w0],
                in0=pt_dst[:, b_lo - w0:b_hi - w0],
                in1=band_sb[:, h, m0:m0 + (b_hi - b_lo)],
            )

    # den segment order: pair*4 + hh*2 + qh  (pair-contiguous for the
    # per-pair normalize batch)
    NSEG = 2 * N_PAIRS * N_QH
    den_sb = nrm.tile([1, NSEG, QH], F32, tag="den")

    def normalize(pair, qh, hh, w0, acc):
        """Evict unnormalized C^T and stash the raw denominator row."""
        nc.vector.tensor_copy(
            out=ct_sb[hh * 64:hh * 64 + 64, pair, w0:w0 + QH],
            in_=acc[0:DK, :])
        if not skip_norm:
            s = pair * 4 + hh * 2 + qh
            nc.vector.tensor_copy(
                out=den_sb[:, s, :],
                in_=acc[DK:DK + 1, :])

    def normalize_pq(pair, qh):
        """Broadcast+reciprocal+divide for one (pair, q-half): 2 segments.

        Each batch runs right after its two evictions, overlapping the rest
        of attention; only the last (pair, qh) batch is a serial tail.
        """
        den_v = dram["den"].ap()
        s0 = pair * 4 + qh
        ssrc = bass.AP(
            tensor=den_sb.tensor, offset=den_sb.offset + s0 * QH,
            ap=[list(den_sb.ap[0])] + [[2 * QH, 2], [1, QH]],
        )
        sdst = bass.AP(
            tensor=den_v.tensor, offset=den_v.offset + s0 * QH,
            ap=[[2 * QH, 2], [1, QH]],
        )
        nc.sync.dma_start(out=sdst, in_=ssrc)
        rbc = nrm.tile([128, QH], F32, tag="rbc")
        for hh in range(2):
            bsrc = bass.AP(
                tensor=den_v.tensor,
                offset=den_v.offset + (s0 + 2 * hh) * QH,
                ap=[[0, 64], [1, QH]],
            )
            nc.sync.dma_start(out=rbc[hh * 64:hh * 64 + 64, :], in_=bsrc)
        nc.vector.reciprocal(out=rbc, in_=rbc)
        nc.vector.tensor_mul(
            out=ct_sb[:, pair, qh * QH:(qh + 1) * QH],
            in0=ct_sb[:, pair, qh * QH:(qh + 1) * QH],
            in1=rbc,
        )

    for pair in range(N_PAIRS):
        for qh in range(N_QH):
            w0 = qh * QH
            if interleave_pv:
                acc_a = accp.tile([DK + 1, QH], F32, tag="acc")
                acc_b = accp.tile([DK + 1, QH], F32, tag="acc")
                accs = [acc_a, acc_b]
                for kc in range(N_KC):
                    pt = ptp.tile([128, 2, QH], PD, tag="pt")
                    for hh in range(2):
                        st = stp.tile([128, QH], F32, tag="st")
                        qk_exp_fix(pair, hh, kc, w0, st, pt[:, hh, :])
                    for hh in range(2):
                        for sub in range(QH // 512):
                            nc.tensor.matmul(
                                accs[hh][:, sub * 512:(sub + 1) * 512],
                                lhsT=v_sb[:, kc, 2 * pair + hh, :],
                                rhs=pt[:, hh, sub * 512:(sub + 1) * 512],
                                start=(kc == 0), stop=(kc == N_KC - 1),
                            )
                for hh in range(2):
                    normalize(pair, qh, hh, w0, accs[hh])
                if not skip_norm:
                    normalize_pq(pair, qh)
            else:
                for kc in range(N_KC):
                    for hh in range(2):
                        st = stp.tile([128, QH], F32, tag="st")
                        qk_exp_fix(pair, hh, kc, w0, st, pt_all[:, hh, kc, :])
                for hh in range(2):
                    acc = accp.tile([DK + 1, QH], F32, tag="acc")
                    for kc in range(N_KC):
                        for sub in range(QH // 512):
                            nc.tensor.matmul(
                                acc[:, sub * 512:(sub + 1) * 512],
                                lhsT=v_sb[:, kc, 2 * pair + hh, :],
                                rhs=pt_all[:, hh, kc, sub * 512:(sub + 1) * 512],
                                start=(kc == 0), stop=(kc == N_KC - 1),
                            )
                    normalize(pair, qh, hh, w0, acc)
                if not skip_norm:
                    normalize_pq(pair, qh)

    attn_ctx.close()

    # ---------- phase 3: Wo partial ----------
    for st_i in range(S // 128):
        ps = stp.tile([128, 1024], F32, tag="st")
        o_sb = outp.tile([128, D], F32, tag="o_sb")
        for mt in range(2):
            for j in range(2):
                nc.tensor.matmul(
                    ps[:, mt * 512:(mt + 1) * 512],
                    lhsT=mm(ct_sb[:, j, st_i * 128:(st_i + 1) * 128]),
                    rhs=mm(wo_sb[:, j, mt * 512:(mt + 1) * 512]),
                    start=(j == 0), stop=(j == 1),
                )
        nc.scalar.copy(out=o_sb, in_=ps)
        nc.sync.dma_start(out=dram["out"].ap()[st_i * 128:(st_i + 1) * 128, :], in_=o_sb)


def make_core_inputs(x, Wq, bq, Wk, bk, Wv, bv, Wo, bo, rel_bias):
    """Host-side shard prep. Returns list of 8 in_maps."""
    x = np.asarray(x, np.float32)
    in_maps = []
    WqT = np.ascontiguousarray(np.asarray(Wq, np.float32).T)
    WkT = np.ascontiguousarray(np.asarray(Wk, np.float32).T)
    WvT = np.ascontiguousarray(np.asarray(Wv, np.float32).T)
    WoT = np.ascontiguousarray(np.asarray(Wo, np.float32).T)
    rel = np.asarray(rel_bias, np.float32)
    xt = [np.ascontiguousarray(x[b].T) for b in range(B)]

    # band multiplier: [p, h_local, m] = exp(bias(q,k) - c_past), q-k = m-128-p
    p_i = np.arange(128)[:, None]
    m_i = np.arange(BAND)[None, :]
    delta = np.clip(m_i - 128 - p_i, -MAX_REL, MAX_REL) + MAX_REL  # [128, 384]

    for c in range(N_CORES):
        b = c // CORES_PER_BATCH
        g = c % CORES_PER_BATCH
        c0 = g * CL
        heads = np.arange(g * HEADS_PER_CORE, (g + 1) * HEADS_PER_CORE)

        bqk = np.empty((128, 4), np.float32)
        bqk[:, 0] = np.asarray(bq, np.float32)[c0:c0 + 128]
        bqk[:, 1] = np.asarray(bq, np.float32)[c0 + 128:c0 + 256]
        bqk[:, 2] = np.asarray(bk, np.float32)[c0:c0 + 128]
        bqk[:, 3] = np.asarray(bk, np.float32)[c0 + 128:c0 + 256]

        band = np.empty((128, HEADS_PER_CORE, BAND), np.float32)
        abias = np.empty((128, 2 * HEADS_PER_CORE), np.float32)
        for i, hg in enumerate(heads):
            c_past = rel[hg, 2 * MAX_REL]
            band[:, i, :] = np.exp(rel[hg][delta] - c_past)
            abias[:, 2 * i] = np.exp(rel[hg, 0] - c_past)  # future multiplier
            abias[:, 2 * i + 1] = c_past
        in_maps.append({
            "xt": xt[b],
            "wqt": np.ascontiguousarray(WqT[:, c0:c0 + CL]),
            "wkt": np.ascontiguousarray(WkT[:, c0:c0 + CL]),
            "wvt": np.ascontiguousarray(WvT[:, c0:c0 + CL]),
            "wot": np.ascontiguousarray(WoT[c0:c0 + CL, :]),
            "bqk": bqk,
            "band": band,
            "abias": abias,
        })
    return in_maps


_NC_CACHE = {}


def get_program(**kw):
    key = tuple(sorted(kw.items()))
    if key not in _NC_CACHE:
        _NC_CACHE[key] = build_program(**kw)
    return _NC_CACHE[key]


def kernel(x, Wq, bq, Wk, bk, Wv, bv, Wo, bo, rel_bias):
    from concourse.bass_utils import run_bass_kernel_spmd

    nc = get_program()
    in_maps = make_core_inputs(x, Wq, bq, Wk, bk, Wv, bv, Wo, bo, rel_bias)
    res = run_bass_kernel_spmd(nc, in_maps, core_ids=list(range(N_CORES)))
    results = res.results

    Wo_np = np.asarray(Wo, np.float32)
    const = np.asarray(bv, np.float32) @ Wo_np.T + np.asarray(bo, np.float32)
    out = np.zeros((B, S, D), np.float32)
    for c in range(N_CORES):
        out[c // CORES_PER_BATCH] += results[c]["out_p"]
    out += const[None, None, :]
    return out



# revision 8
# speedup vs baseline: 1.3505x; 1.3505x over previous
"""Trainium2 Bass kernel for MultiHeadAttention with relative position bias.

Reference computation (B=2, S=2048, D=1024, H=16, Dk=64, MAX_REL=128):
    Q,K,V = x@W{q,k,v}.T + b      (per-head reshape)
    scores = QK^T/sqrt(Dk) + rel_bias_matrix
    out = softmax(scores) @ V, heads merged, @ Wo.T + bo

Sharding (8 cores): core c handles batch b=c//4 and 4 heads hg=4*(c%4)..+4
(data + head parallel). Q/K/V projections column-split per head group,
Wo row-split; the partial outputs are summed on the host (the "all-reduce").

Per-core device algorithm (all matmul operands bf16; f32 PSUM accumulate):
  xT (1024,2048) -> Q^T,K^T (c_local=256, S) on PE; V as (S, 256).
  Per head pair (row-tiled 64x128 PE, two heads concurrent):
    S^T[k,q] = K^T.T @ Q^T, then P^T = exp(S^T/8) via one ACT pass; the
    "future" region (q-k <= -128) is fixed with a constant multiply and the
    384-wide Toeplitz band with a host-precomputed exp(bias - c_past) tile
    (both DVE, bf16).  P^T stored bf16.
  PV: lhsT is the M=128 augmented [V_h0 | ones] (hh=0) / [ones | V_h1]
  (hh=1), so acc rows carry the head output AND the softmax denominator
  replicated across 64 partitions (PV matmuls are N-bound, so this is free).
  Normalize: two partition-shifting PSUM->SBUF DMAs collect both heads'
  denominators into one [128,QH] tile, reciprocal_approx_fast (DVE), then
  two fused evict-normalize multiplies into bf16 ct.
  Wo partial = ct.T @ (Wo^T rows), per-q normalization already applied.

Issue order maximizes ACT (exp) occupancy -- the hard wall is
4 heads x S^2 = 16.8M exps/core on the scalar engine (~110us):
K flight, Q(t0), V flight, attention qh0 (both pairs), Q(t1) interlude,
attention qh1, Wo (first half overlapped with the last normalize).
"""

import math
import os
import sys

for _p in ("/opt/trn_rl_repo", "/root/.axon_site", "/root/.axon_site/_ro/trn_rl_repo",
           "/root/.axon_site/_ro/pypackages"):
    if os.path.isdir(_p) and _p not in sys.path:
        sys.path.append(_p)

import numpy as np
import ml_dtypes

import concourse.bass as bass
import concourse.mybir as mybir
import concourse.tile as tile
from concourse import bacc
from contextlib import ExitStack

# Problem constants (hardcoded per the contract).
B, S, D = 2, 2048, 1024
H, DK = 16, 64
MAX_REL = 128
N_CORES = 8
CORES_PER_BATCH = 4
HEADS_PER_CORE = H // CORES_PER_BATCH  # 4
CL = HEADS_PER_CORE * DK               # 256 local channels
N_PAIRS = HEADS_PER_CORE // 2          # 2 head pairs
QH = 1024                              # q processed in halves
N_QH = S // QH                         # 2
N_KC = S // 128                        # 16 k chunks
BAND = 3 * 128                         # band width in q for one k chunk
NDC = D // 128                         # 8 contraction chunks

F32 = mybir.dt.float32
BF16 = mybir.dt.bfloat16

SCALE = 1.0 / math.sqrt(DK)

EXP = mybir.ActivationFunctionType.Exp


def build_program(reps=1):
    nc = bacc.Bacc("TRN2", target_bir_lowering=False, debug=False)

    xt_d = nc.declare_dram_parameter("xt", [D, S], BF16, isOutput=False)
    wqt_d = nc.declare_dram_parameter("wqt", [D, CL], BF16, isOutput=False)
    wkt_d = nc.declare_dram_parameter("wkt", [D, CL], BF16, isOutput=False)
    wvt_d = nc.declare_dram_parameter("wvt", [D, CL], BF16, isOutput=False)
    wot_d = nc.declare_dram_parameter("wot", [CL, D], BF16, isOutput=False)
    bqk_d = nc.declare_dram_parameter("bqk", [128, 4], F32, isOutput=False)
    band_d = nc.declare_dram_parameter("band", [128, HEADS_PER_CORE, BAND], BF16,
                                       isOutput=False)
    # per-head activation constants, replicated over partitions:
    # [:, 2h] = exp(c_fut - c_past) multiplier
    abias_d = nc.declare_dram_parameter("abias", [128, 2 * HEADS_PER_CORE], F32,
                                        isOutput=False)
    out_d = nc.declare_dram_parameter("out_p", [S, D], F32, isOutput=True)

    with tile.TileContext(nc) as tc, ExitStack() as ctx:
        # ---------- long-lived SBUF ----------
        persist = ctx.enter_context(tc.tile_pool(name="persist", bufs=1))
        q_sb = persist.tile([128, 2, S], BF16, tag="q_sb")
        k_sb = persist.tile([128, 2, S], BF16, tag="k_sb")
        # per (kc, pair): [V_h0(64) | ones(128) | V_h1(64)]
        v_sb = persist.tile([128, N_KC, N_PAIRS, 256], BF16, tag="v_sb")
        ct_sb = persist.tile([128, 2, S], BF16, tag="ct_sb")
        wo_sb = persist.tile([128, 2, D], BF16, tag="wo_sb")
        band_sb = persist.tile([128, HEADS_PER_CORE, BAND], BF16, tag="band_sb")
        bqk_sb = persist.tile([128, 4], F32, tag="bqk_sb")
        abias_sb = persist.tile([128, 2 * HEADS_PER_CORE], F32, tag="abias_sb")

        xw = ctx.enter_context(tc.tile_pool(name="xw", bufs=1))
        xt_sb = xw.tile([128, NDC, S], BF16, tag="xt_sb")
        wq_sb = xw.tile([128, NDC, CL], BF16, tag="wq_sb")
        wk_sb = xw.tile([128, NDC, CL], BF16, tag="wk_sb")
        wv_sb = xw.tile([128, NDC, CL], BF16, tag="wv_sb")

        # ---------- PSUM pools ----------
        stp = ctx.enter_context(tc.tile_pool(name="stp", bufs=2, space="PSUM"))
        accp = ctx.enter_context(tc.tile_pool(name="accp", bufs=2, space="PSUM"))

        # ---------- small pools ----------
        outp = ctx.enter_context(tc.tile_pool(name="outp", bufs=4))
        nrm = ctx.enter_context(tc.tile_pool(name="nrm", bufs=4))
        ptp = ctx.enter_context(tc.tile_pool(name="ptp", bufs=6))

        sb = dict(q=q_sb, k=k_sb, v=v_sb, ct=ct_sb, wo=wo_sb, band=band_sb,
                  bqk=bqk_sb, abias=abias_sb, xt=xt_sb, wq=wq_sb, wk=wk_sb,
                  wv=wv_sb)
        dram = dict(xt=xt_d, wqt=wqt_d, wkt=wkt_d, wvt=wvt_d, wot=wot_d,
                    bqk=bqk_d, band=band_d, abias=abias_d, out=out_d)
        pools = dict(stp=stp, accp=accp, outp=outp, nrm=nrm, ptp=ptp)

        for rep in range(reps):
            _phases(nc, tc, sb, dram, pools, rep)

    nc.compile()
    return nc


def _phases(nc, tc, sb, dram, pools, rep):
    q_sb, k_sb, v_sb, ct_sb, wo_sb = sb["q"], sb["k"], sb["v"], sb["ct"], sb["wo"]
    band_sb, bqk_sb, abias_sb = sb["band"], sb["bqk"], sb["abias"]
    xt_sb, wq_sb, wk_sb, wv_sb = sb["xt"], sb["wq"], sb["wk"], sb["wv"]
    stp, accp, outp, nrm, ptp = (pools[n] for n in
                                 ("stp", "accp", "outp", "nrm", "ptp"))

    GROUPS = (range(0, NDC // 2), range(NDC // 2, NDC))
    xt_v = dram["xt"].ap().rearrange("(c p) s -> p c s", p=128)

    # ---------- input DMAs, ordered so K-flight group 0 starts earliest ----
    nc.sync.dma_start(out=wk_sb, in_=dram["wkt"].ap().rearrange("(c p) m -> p c m", p=128))
    nc.sync.dma_start(out=bqk_sb, in_=dram["bqk"].ap())
    for dc in range(NDC // 2):
        nc.sync.dma_start(out=xt_sb[:, dc, :], in_=xt_v[:, dc, :])
    nc.sync.dma_start(out=wq_sb, in_=dram["wqt"].ap().rearrange("(c p) m -> p c m", p=128))
    for dc in range(NDC // 2, NDC):
        nc.sync.dma_start(out=xt_sb[:, dc, :], in_=xt_v[:, dc, :])
    nc.sync.dma_start(out=wv_sb, in_=dram["wvt"].ap().rearrange("(c p) m -> p c m", p=128))
    nc.sync.dma_start(out=wo_sb, in_=dram["wot"].ap().rearrange("(c p) m -> p c m", p=128))
    nc.sync.dma_start(out=abias_sb, in_=dram["abias"].ap())
    nc.sync.dma_start(out=band_sb, in_=dram["band"].ap())
    # ones blocks of the augmented V (middle 128 columns of each pair block)
    nc.vector.memset(v_sb[:, :, :, 64:192], 1.0)

    # ---------- K flight: 4 concurrent PSUM slots, as in the classic flight --
    def qk_flight_full(w_sb, o_sb, boff):
        slots = [stp.tile([128, 1024], F32, tag="st", name="fl_s0"),
                 stp.tile([128, 1024], F32, tag="st", name="fl_s1"),
                 accp.tile([128, 1024], F32, tag="acc", name="fl_s2"),
                 accp.tile([128, 1024], F32, tag="acc", name="fl_s3")]
        for g in GROUPS:
            for j in range(2):
                for t in range(2):
                    ps = slots[j * 2 + t]
                    for half in range(2):
                        for dc in g:
                            nc.tensor.matmul(
                                ps[:, half * 512:(half + 1) * 512],
                                lhsT=w_sb[:, dc, j * 128:(j + 1) * 128],
                                rhs=xt_sb[:, dc, t * 1024 + half * 512:
                                          t * 1024 + (half + 1) * 512],
                                start=(dc == 0), stop=(dc == NDC - 1),
                            )
        for j in range(2):
            for t in range(2):
                nc.scalar.add(
                    out=o_sb[:, j, t * 1024:(t + 1) * 1024],
                    in_=slots[j * 2 + t],
                    add=bqk_sb[:, boff + j:boff + j + 1],
                )

    def q_half_flight(t, slot_a, slot_b):
        """Q projection for q window [t*1024, (t+1)*1024), both channel js."""
        slots = (slot_a, slot_b)
        for g in GROUPS:
            for j in range(2):
                ps = slots[j]
                for half in range(2):
                    for dc in g:
                        nc.tensor.matmul(
                            ps[:, half * 512:(half + 1) * 512],
                            lhsT=wq_sb[:, dc, j * 128:(j + 1) * 128],
                            rhs=xt_sb[:, dc, t * 1024 + half * 512:
                                      t * 1024 + (half + 1) * 512],
                            start=(dc == 0), stop=(dc == NDC - 1),
                        )
        for j in range(2):
            nc.scalar.add(
                out=q_sb[:, j, t * 1024:(t + 1) * 1024],
                in_=slots[j],
                add=bqk_sb[:, j:j + 1],
            )

    qk_flight_full(wk_sb, k_sb, 2)
    q_half_flight(0, stp.tile([128, 1024], F32, tag="st", name="q0_a"),
                  accp.tile([128, 1024], F32, tag="acc", name="q0_b"))

    # ---------- V flight: [s_chunk, dv], 4 s-chunks packed per 2 PSUM slots --
    for scg in range(N_KC // 4):
        ps = stp.tile([128, 1024], F32, tag="st")
        psb = accp.tile([128, 1024], F32, tag="acc")
        both = (ps, psb)
        for g in GROUPS:
            for i in range(4):
                sc = scg * 4 + i
                tgt = both[i // 2]
                col = (i % 2) * 512
                for dc in g:
                    nc.tensor.matmul(
                        tgt[:, col:col + CL],
                        lhsT=xt_sb[:, dc, sc * 128:(sc + 1) * 128],
                        rhs=wv_sb[:, dc, :],
                        start=(dc == 0), stop=(dc == NDC - 1),
                    )
        for i in range(4):
            sc = scg * 4 + i
            tgt = both[i // 2]
            col = (i % 2) * 512
            # psum col layout: h*64+d for h in 0..3; pair hp = h//2.
            src = tgt[:, col:col + CL].rearrange("p (hp dd) -> p hp dd", hp=2)
            # even heads -> cols 0:64 of each pair block
            nc.vector.tensor_copy(out=v_sb[:, sc, :, 0:64], in_=src[:, :, 0:64])
            # odd heads -> cols 192:256 of each pair block
            nc.vector.tensor_copy(out=v_sb[:, sc, :, 192:256], in_=src[:, :, 64:128])

    # ---------- attention ----------
    def qk_exp_fix(pair, hh, kc, w0, st, pt_dst):
        """QK matmuls + exp + band/future fixups for one (head, chunk)."""
        k0 = kc * 128
        h = 2 * pair + hh
        p0 = hh * 64
        for half in range(QH // 512):
            nc.tensor.matmul(
                st[:, half * 512:(half + 1) * 512],
                lhsT=k_sb[p0:p0 + 64, pair, k0:k0 + 128],
                rhs=q_sb[p0:p0 + 64, pair,
                         w0 + half * 512:w0 + (half + 1) * 512],
                start=True, stop=True,
                tile_position=(p0, 0),
            )
        nc.scalar.activation(out=pt_dst, in_=st, func=EXP, scale=SCALE)
        # future region (q <= k0-129): multiply by exp(c_fut - c_past)
        fut_end = min(max(k0 - 128, w0), w0 + QH)
        n_fut = fut_end - w0
        if n_fut > 0:
            nc.vector.tensor_scalar_mul(
                out=pt_dst[:, 0:n_fut], in0=pt_dst[:, 0:n_fut],
                scalar1=abias_sb[:, 2 * h:2 * h + 1],
            )
        # band: q in [k0-128, k0+256) -> multiply exp(bias - c_past)
        b_lo = max(k0 - 128, w0)
        b_hi = min(k0 + 2 * 128, w0 + QH)
        if b_hi > b_lo:
            m0 = b_lo - (k0 - 128)
            nc.vector.tensor_mul(
                out=pt_dst[:, b_lo - w0:b_hi - w0],
                in0=pt_dst[:, b_lo - w0:b_hi - w0],
                in1=band_sb[:, h, m0:m0 + (b_hi - b_lo)],
            )

    def attn_block(pair, qh):
        """One (head-pair, q-half): QK+exp+fix, PV accumulate, normalize.

        acc_a (hh=0) rows: 0:64 = ct_h0 unnormalized, 64:128 = den_h0 x64.
        acc_b (hh=1) rows: 0:64 = den_h1 x64, 64:128 = ct_h1 unnormalized.
        """
        w0 = qh * QH
        acc_a = accp.tile([128, QH], F32, tag="acc")
        acc_b = accp.tile([128, QH], F32, tag="acc")
        accs = [acc_a, acc_b]
        for kc in range(N_KC):
            pt = ptp.tile([128, 2, QH], BF16, tag="pt")
            for hh in range(2):
                st = stp.tile([128, QH], F32, tag="st")
                qk_exp_fix(pair, hh, kc, w0, st, pt[:, hh, :])
            for hh in range(2):
                for sub in range(QH // 512):
                    nc.tensor.matmul(
                        accs[hh][:, sub * 512:(sub + 1) * 512],
                        lhsT=v_sb[:, kc, pair, hh * 128:(hh + 1) * 128],
                        rhs=pt[:, hh, sub * 512:(sub + 1) * 512],
                        start=(kc == 0), stop=(kc == N_KC - 1),
                    )
        # normalize: partition-shifted PSUM->SBUF copies collect both heads'
        # denominators (reciprocal_approx_fast can't read PSUM), then an
        # all-SBUF approx-reciprocal and fused evict-normalize multiplies.
        den = nrm.tile([128, QH], F32, tag="den")
        rden = nrm.tile([128, QH], F32, tag="rden")
        nc.vector.tensor_copy(out=den[0:64, :], in_=acc_a[64:128, :])
        nc.vector.tensor_copy(out=den[64:128, :], in_=acc_b[0:64, :])
        nc.vector.reciprocal_approx_fast(out=rden, in_=den)
        nc.vector.tensor_mul(
            out=ct_sb[0:64, pair, w0:w0 + QH],
            in0=acc_a[0:64, :], in1=rden[0:64, :],
        )
        nc.vector.tensor_mul(
            out=ct_sb[64:128, pair, w0:w0 + QH],
            in0=acc_b[64:128, :], in1=rden[64:128, :],
        )

    def wo_chunk(st_i):
        ps = stp.tile([128, 1024], F32, tag="st")
        o_sb = outp.tile([128, D], F32, tag="o_sb")
        for mt in range(2):
            for j in range(2):
                nc.tensor.matmul(
                    ps[:, mt * 512:(mt + 1) * 512],
                    lhsT=ct_sb[:, j, st_i * 128:(st_i + 1) * 128],
                    rhs=wo_sb[:, j, mt * 512:(mt + 1) * 512],
                    start=(j == 0), stop=(j == 1),
                )
        nc.scalar.copy(out=o_sb, in_=ps)
        nc.sync.dma_start(out=dram["out"].ap()[st_i * 128:(st_i + 1) * 128, :],
                          in_=o_sb)

    # qh-major so Q(t1) can slot in between, and Wo qh0 half is unblocked early
    attn_block(0, 0)
    attn_block(1, 0)
    q_half_flight(1, accp.tile([128, 1024], F32, tag="acc", name="q1_a"),
                  accp.tile([128, 1024], F32, tag="acc", name="q1_b"))
    attn_block(0, 1)
    attn_block(1, 1)
    # Wo: first half executes during the last block's normalize chain
    for st_i in range(S // 128):
        wo_chunk(st_i)


def make_core_inputs(x, Wq, bq, Wk, bk, Wv, bv, Wo, bo, rel_bias):
    """Host-side shard prep. Returns list of 8 in_maps."""
    bf16 = ml_dtypes.bfloat16
    x = np.asarray(x, np.float32)
    in_maps = []
    WqT = np.ascontiguousarray(np.asarray(Wq, np.float32).T)
    WkT = np.ascontiguousarray(np.asarray(Wk, np.float32).T)
    WvT = np.ascontiguousarray(np.asarray(Wv, np.float32).T)
    WoT = np.ascontiguousarray(np.asarray(Wo, np.float32).T)
    rel = np.asarray(rel_bias, np.float32)
    xt = [np.ascontiguousarray(x[b].T).astype(bf16) for b in range(B)]

    # band multiplier: [p, h_local, m] = exp(bias(q,k) - c_past), q-k = m-128-p
    p_i = np.arange(128)[:, None]
    m_i = np.arange(BAND)[None, :]
    delta = np.clip(m_i - 128 - p_i, -MAX_REL, MAX_REL) + MAX_REL  # [128, 384]

    for c in range(N_CORES):
        b = c // CORES_PER_BATCH
        g = c % CORES_PER_BATCH
        c0 = g * CL
        heads = np.arange(g * HEADS_PER_CORE, (g + 1) * HEADS_PER_CORE)

        bqk = np.empty((128, 4), np.float32)
        bqk[:, 0] = np.asarray(bq, np.float32)[c0:c0 + 128]
        bqk[:, 1] = np.asarray(bq, np.float32)[c0 + 128:c0 + 256]
        bqk[:, 2] = np.asarray(bk, np.float32)[c0:c0 + 128]
        bqk[:, 3] = np.asarray(bk, np.float32)[c0 + 128:c0 + 256]

        band = np.empty((128, HEADS_PER_CORE, BAND), np.float32)
        abias = np.empty((128, 2 * HEADS_PER_CORE), np.float32)
        for i, hg in enumerate(heads):
            c_past = rel[hg, 2 * MAX_REL]
            band[:, i, :] = np.exp(rel[hg][delta] - c_past)
            abias[:, 2 * i] = np.exp(rel[hg, 0] - c_past)  # future multiplier
            abias[:, 2 * i + 1] = c_past
        in_maps.append({
            "xt": xt[b],
            "wqt": np.ascontiguousarray(WqT[:, c0:c0 + CL]).astype(bf16),
            "wkt": np.ascontiguousarray(WkT[:, c0:c0 + CL]).astype(bf16),
            "wvt": np.ascontiguousarray(WvT[:, c0:c0 + CL]).astype(bf16),
            "wot": np.ascontiguousarray(WoT[c0:c0 + CL, :]).astype(bf16),
            "bqk": bqk,
            "band": band.astype(bf16),
            "abias": abias,
        })
    return in_maps


_NC_CACHE = {}


def get_program(**kw):
    key = tuple(sorted(kw.items()))
    if key not in _NC_CACHE:
        _NC_CACHE[key] = build_program(**kw)
    return _NC_CACHE[key]


def kernel(x, Wq, bq, Wk, bk, Wv, bv, Wo, bo, rel_bias):
    from concourse.bass_utils import run_bass_kernel_spmd

    nc = get_program()
    in_maps = make_core_inputs(x, Wq, bq, Wk, bk, Wv, bv, Wo, bo, rel_bias)
    res = run_bass_kernel_spmd(nc, in_maps, core_ids=list(range(N_CORES)))
    results = res.results

    Wo_np = np.asarray(Wo, np.float32)
    const = np.asarray(bv, np.float32) @ Wo_np.T + np.asarray(bo, np.float32)
    out = np.zeros((B, S, D), np.float32)
    for c in range(N_CORES):
        out[c // CORES_PER_BATCH] += results[c]["out_p"]
    out += const[None, None, :]
    return out
